# revision 8
# baseline (speedup 1.0000x reference)
"""MMD loss kernel for Trainium2, SPMD across 8 NeuronCores.

Math: loss = (1/B^2) * sum_{ij} s_i s_j K_ij over the [2B, 2B] Gaussian
kernel-sum matrix, s = [+1]*B ++ [-1]*B.  K_ij = sum_{k=0..4} exp(-l2_ij / (bw*2^k))
with bw = mean off-diagonal l2 / 4 (computed on host via the algebraic identity
sum(l2) = 2N*sum(sq) - 2*||sum x||^2).

Device strategy per core (SPMD, identical program; per-core data sliced on host):
  - 16x16 grid of 512-wide blocks over the symmetric 8192x8192 matrix.
    Core c owns block-rows {2c, 2c+1}. 17 blocks per core: (r0, d=0..7),
    (r1, d=0..7) and ONE wrap-diagonal d=8 block at weight 2 (cores 0-3 take
    the even row's pair, cores 4-7 the odd row's; v slots 9/10 hold its data
    so the program is identical across cores). d=0 weight 1, d=1..7 weight 2.
  - PSUM accumulates P = x_i.x_j - (sq_i+sq_j)/2 = -l2/2 via 4 fp8e4m3
    DoubleRow matmuls (kd-pair contraction, 2x PE rate: ~216ns warm at N=512)
    plus one K=2 fp32r matmul adding the exact -(sq_i+sq_j)/2 (strip-packed:
    4 concurrent row-group MMs via tile_position, ~470ns).
  - Elementwise: with t1 = exp(2c4*P), the five kernel levels are
    t1^(2^k), k=0..4.  ONE ACT exp produces t1 (bf16) with accum -> S1;
    ONE custom DVE op (POWSUM8_ANT: x^2+x^4+x^8+x^16, accum=ADD, runs in
    the fp32 datapath) produces S2+S4+S8+S16 in a single 1x pass (~2.3us).
    Steady state is PE-bound at ~4us/block (ACT ~2.3us, DVE ~2.4us busy).
  - Head: aug/sc DMAs are issued between the v0 chunks (tiny transfers) so
    block 0's aug matmuls don't wait; bridge warmup matmuls keep the PE
    clock at 2.4GHz across the first-DMA latency window.
  - Tail: the last block runs in column quarters ping-ponging between the
    two psum buffers so the post-matmul ACT/DVE drain pipelines.
  - DMA: whole-slot 512KB transfers (sync engine pays ~600ns per
    DMA_DIRECT2D regardless of size).
  - Host reduces the per-core [128, 40] sums with block weights/signs.
"""

import sys

sys.path.insert(0, "/opt/trn_rl_repo")

import numpy as np
import ml_dtypes

import concourse.mybir as mybir
import concourse.tile as tile
from concourse import bacc
from concourse.bass_utils import run_bass_kernel_spmd

B = 4096
D = 1024
N = 2 * B
NB = 16          # block grid (512-wide)
BS = 512
KD = 8           # feature k-tiles of 128
G = 4            # kd pairs (DoubleRow groups)
NCORES = 8
NVSLOT = 11      # v slots: 0..8 consecutive cols, 9/10 the wrap-pair block
NBLK = 17
NSLOT = 2 * 16 + 2 * 2   # [S1, Spow] per full block; last block halved

FP8 = mybir.dt.float8e4
BF16 = mybir.dt.bfloat16
F32 = mybir.dt.float32
F32R = mybir.dt.float32r
NP_FP8 = ml_dtypes.float8_e4m3

# program block list: (lhsT slot, rhs slot)
BLOCKS = [(0, d) for d in range(8)] + [(1, 1 + d) for d in range(8)] + [(9, 10)]
UACOL = {0: 0, 1: 1, 9: 2}   # lhsT slot -> aug column holding -sq/2

_prog_cache = {}


def register_powsum():
    """Register POWSUM8_ANT (x^2+x^4+x^8+x^16, accum=ADD) in dve_ops.OPS.

    The uops sha is computed at registration so the pin check always
    matches this toolchain's lower() output."""
    import concourse.dve_ops as dve_ops
    from concourse.dve_spec import Spec, Src0, sq, lower, AluOp
    from concourse.dve_uop import DveOpSpec

    for o in dve_ops.OPS:
        if o.name == "POWSUM8_ANT":
            return o

    x = Src0
    m1 = sq(x)
    m2 = sq(m1)
    m3 = sq(m2)
    m4 = sq(m3)

    def ref(in0, in1, s0, s1, imm2):
        b = in0.astype(np.float32)
        b2 = b * b
        b4 = b2 * b2
        b8 = b4 * b4
        b16 = b8 * b8
        o = (b2 + b4 + b8 + b16).astype(np.float32)
        return o, o.reshape(o.shape[0], -1).sum(-1, keepdims=True)

    spec = Spec(body=m1 + m2 + m3 + m4, accum=AluOp.ADD, reference=ref)
    name = "POWSUM8_ANT"
    row = dve_ops._CUSTOM_DVE_ROW_BASE + len(dve_ops.OPS)
    assert row < 0x20
    shas = {}
    for ver in ("v3", "v4"):
        try:
            u = lower(spec, ver=ver)
            shas[ver] = DveOpSpec(name=name, opcode=row, uops=u, rd1_en=False).sha(ver)
        except Exception:
            pass
    op = dve_ops.DveOp(name, spec, subdim=False, uops_sha=shas)
    dve_ops.OPS.append(op)
    dve_ops._SUB_OPCODE_FOR_NAME[name] = row
    dve_ops.CUSTOM_DVE_SPECS[name] = spec
    return op


def build_program():
    if "nc" in _prog_cache:
        return _prog_cache["nc"]
    powsum = register_powsum()
    nc = bacc.Bacc("TRN2", target_bir_lowering=False, debug=False, num_devices=NCORES)
    v_d = nc.dram_tensor("v", [NVSLOT, 128, KD, BS], FP8, kind="ExternalInput").ap()
    aug_d = nc.dram_tensor("aug", [8, 14, BS], F32R, kind="ExternalInput").ap()
    sc_d = nc.dram_tensor("sc", [128, 3], F32, kind="ExternalInput").ap()
    out_d = nc.dram_tensor("out", [128, NSLOT], F32, kind="ExternalOutput").ap()

    EXP = mybir.ActivationFunctionType.Exp
    DR = mybir.MatmulPerfMode.DoubleRow

    with tile.TileContext(nc) as tc:
        with (
            tc.tile_pool(name="vstat", bufs=1) as vpool,
            tc.tile_pool(name="augp", bufs=1) as augpool,
            tc.tile_pool(name="cst", bufs=1) as cstpool,
            tc.tile_pool(name="ot", bufs=1) as opool,
            tc.tile_pool(name="texp", bufs=3) as tpool,
            tc.tile_pool(name="wsq", bufs=2) as wpool,
            tc.tile_pool(name="ps", bufs=2, space="PSUM") as pspool,
        ):
            v_sb = vpool.tile([128, NVSLOT, KD, BS], FP8)
            aug_sb = augpool.tile([128, 14, BS], F32R)
            sc_sb = cstpool.tile([128, 3], F32)
            out_sb = opool.tile([128, NSLOT], F32)

            # DMA order == consumption order, with one trap: the aug strips'
            # strided dst partitions (32g+k) all map to DMA engine k (p%16),
            # so each aug transfer serializes onto ONE engine (~6us for
            # 114KB). Split each into a head piece (cols 0:4 = all three ua
            # rows + ones/va slot 0 -> everything block 0 touches, ~32KB,
            # ~2us) right after v0 chunk 0, and the remainder (cols 4:14)
            # after v1 (block b first touches col 3+b at ~14+4b us).
            # aug strips: partitions 32g+0 get (ua=-sq/2, va=+1) rows,
            # partitions 32g+1 get (ua=-1, va=sq/2) rows.
            nc.sync.dma_start(
                out=v_sb[:, 0, 0:2, :], in_=v_d[0, :, 0:2, :],
            )
            nc.sync.dma_start(out=aug_sb[0:128:32, 0:4, :], in_=aug_d[0:4, 0:4, :])
            nc.sync.dma_start(out=aug_sb[1:128:32, 0:4, :], in_=aug_d[4:8, 0:4, :])
            nc.sync.dma_start(out=sc_sb[:], in_=sc_d[:])
            for g in range(1, G):
                nc.sync.dma_start(
                    out=v_sb[:, 0, 2 * g:2 * g + 2, :],
                    in_=v_d[0, :, 2 * g:2 * g + 2, :],
                )
            for h in range(2):
                nc.sync.dma_start(
                    out=v_sb[:, 1, 4 * h:4 * h + 4, :],
                    in_=v_d[1, :, 4 * h:4 * h + 4, :],
                )
            nc.sync.dma_start(out=aug_sb[0:128:32, 4:14, :], in_=aug_d[0:4, 4:14, :])
            nc.sync.dma_start(out=aug_sb[1:128:32, 4:14, :], in_=aug_d[4:8, 4:14, :])
            for s in range(2, NVSLOT):
                nc.sync.dma_start(out=v_sb[:, s], in_=v_d[s])

            # PE warm-up: 56 junk matmuls (as tuned previously) plus a few
            # larger bridge matmuls that keep the HAM activity window busy
            # until the first v0 chunk lands (~10.2us); otherwise the clock
            # re-gates to 1.2GHz and the first two blocks run at half rate.
            # Single upfront memset: a second memset after the small warmups
            # would WAR-depend on all of them and stall the bridge ~1.4us.
            wtile = cstpool.tile([128, 512], BF16)
            nc.gpsimd.memset(wtile[:], 0.25)
            ps_w = pspool.tile([128, 4, BS], F32, name="ps_warm", tag="ps")
            for _ in range(56):
                nc.tensor.matmul(
                    ps_w[0:8, 0, 0:8],
                    lhsT=wtile[:, 0:8],
                    rhs=wtile[:, 0:8],
                    start=True,
                    stop=True,
                )
            # bridge warmups: span the remaining first-DMA latency window
            # (~8.9-11.5us at the mid-clock 427ns each) so the PE clock
            # doesn't re-gate before real data lands.
            for _ in range(6):
                nc.tensor.matmul(
                    ps_w[0:8, 0, :],
                    lhsT=wtile[:, 0:8],
                    rhs=wtile[:, :],
                    start=True,
                    stop=True,
                )

            def emit_mms(ps, lb, rb, cs, cp):
                for g in range(G):
                    for it in range(4):
                        nc.tensor.matmul(
                            ps[:, it, cp],
                            lhsT=v_sb[:, lb, 2 * g:2 * g + 2, it * 128:(it + 1) * 128],
                            rhs=v_sb[:, rb, 2 * g:2 * g + 2, cs],
                            start=(g == 0),
                            stop=False,
                            perf_mode=DR,
                        )
                for it in range(4):
                    nc.tensor.matmul(
                        ps[:, it, cp],
                        lhsT=aug_sb[32 * it:32 * it + 2, UACOL[lb], it * 128:(it + 1) * 128],
                        rhs=aug_sb[32 * it:32 * it + 2, 3 + rb, cs],
                        start=False,
                        stop=True,
                        tile_position=(32 * it, 0),
                    )

            def emit_elem(ps, t1, junk, cs, cp, sbase):
                # t1 = exp(2*c4*P), accum -> S1; POWSUM8 -> S2+S4+S8+S16
                nc.scalar.activation(
                    t1[:, :, cs], ps[:, :, cp], EXP,
                    scale=sc_sb[:, 0:1],
                    accum_out=out_sb[:, sbase:sbase + 1],
                )
                nc.vector._custom_dve(
                    powsum,
                    out=junk[:, :, cs],
                    in0=t1[:, :, cs],
                    accum_out=out_sb[:, sbase + 1:sbase + 2],
                )

            full = slice(0, BS)
            for b, (lb, rb) in enumerate(BLOCKS):
                last = b == NBLK - 1
                t1 = tpool.tile([128, 4, BS], BF16, name=f"t1_{b}", tag="t1")
                junk = wpool.tile([128, 4, BS], BF16, name=f"junk_{b}", tag="junk")
                if not last:
                    ps = pspool.tile([128, 4, BS], F32, name=f"ps_{b}", tag="ps")
                    emit_mms(ps, lb, rb, full, full)
                    emit_elem(ps, t1, junk, full, full, 2 * b)
                else:
                    # full-width matmuls (quartered matmuls are LDWEIGHTS-
                    # bound and slower); only the elementwise drain is halved
                    # so ACT/DVE pipeline the tail.
                    ps = pspool.tile([128, 4, BS], F32, name=f"ps_{b}", tag="ps")
                    emit_mms(ps, lb, rb, full, full)
                    for h in range(2):
                        hs = slice(h * 256, (h + 1) * 256)
                        emit_elem(ps, t1, junk, hs, hs, 32 + 2 * h)
                if b == 15:
                    nc.sync.dma_start(out=out_d[:, 0:32], in_=out_sb[:, 0:32])
            nc.sync.dma_start(out=out_d[:, 32:NSLOT], in_=out_sb[:, 32:NSLOT])
    nc.compile()
    _prog_cache["nc"] = nc
    return nc


def core_slots(c):
    """Global 512-col block indices held by v slots 0..10 on core c."""
    a0 = 2 * c
    slots = [(a0 + s) % NB for s in range(9)]
    if c < 4:
        slots += [a0 % NB, (a0 + 8) % NB]
    else:
        slots += [(a0 + 1) % NB, (a0 + 9) % NB]
    return slots


def prepare_inputs(source: np.ndarray, target: np.ndarray):
    """Host-side shard prep. Returns in_maps for the 8 cores."""
    total = np.concatenate([source, target], axis=0).astype(np.float32)  # [N, D]
    t64 = total.astype(np.float64)
    sq64 = np.einsum("nd,nd->n", t64, t64)
    S1 = sq64.sum()
    vsum = t64.sum(axis=0)
    sum_l2 = 2.0 * N * S1 - 2.0 * (vsum @ vsum)
    bandwidth = sum_l2 / (N * N - N)
    bandwidth = bandwidth / (2.0 ** (5 // 2))  # KERNEL_MUL ** (KERNEL_NUM // 2)
    c4 = np.float64(1.0) / (16.0 * bandwidth)

    sq32 = sq64.astype(np.float32)
    Tt = np.ascontiguousarray(total.T)  # [D, N] f32
    v_all = Tt.astype(NP_FP8).reshape(KD, 128, N)

    # psum holds -l2/2, so the exp scale is doubled vs the -l2 formulation
    sc_np = np.empty((128, 3), dtype=np.float32)
    sc_np[:, 0] = np.float32(2.0 * c4)
    sc_np[:, 1] = np.float32(4.0 * c4)
    sc_np[:, 2] = np.float32(8.0 * c4)

    in_maps = []
    for c in range(NCORES):
        slots = core_slots(c)
        v_np = np.empty((NVSLOT, 128, KD, BS), dtype=NP_FP8)
        aug_np = np.zeros((8, 14, BS), dtype=np.float32)
        for s, gcol in enumerate(slots):
            cols = slice(gcol * BS, (gcol + 1) * BS)
            v_np[s] = v_all[:, :, cols].transpose(1, 0, 2)
        for g in range(4):
            for li, sl in enumerate([slots[0], slots[1], slots[9]]):
                aug_np[g, li] = -0.5 * sq32[sl * BS:(sl + 1) * BS]
            aug_np[g, 3:14] = 1.0
            aug_np[4 + g, 0:3] = -1.0
            for s, gcol in enumerate(slots):
                aug_np[4 + g, 3 + s] = 0.5 * sq32[gcol * BS:(gcol + 1) * BS]
        in_maps.append({"v": v_np, "aug": aug_np, "sc": sc_np})
    return in_maps


def reduce_outputs(outs):
    """outs: list of [128, NSLOT] f32 per core -> loss (np.float32 scalar)."""
    S = 0.0
    for c in range(NCORES):
        o = outs[c].astype(np.float64)  # [128, NSLOT]
        cols = o.sum(axis=0)  # [NSLOT]
        slots = core_slots(c)
        for b, (lb, rb) in enumerate(BLOCKS):
            if b < 16:
                bases = [2 * b]
            else:
                bases = [32 + 2 * h for h in range(2)]
            bsum = sum(cols[base] + cols[base + 1] for base in bases)
            grow = slots[lb]
            gcol = slots[rb]
            w = 1.0 if (b == 0 or b == 8) else 2.0
            sr = 1.0 if grow < NB // 2 else -1.0
            sg = 1.0 if gcol < NB // 2 else -1.0
            S += w * sr * sg * bsum
    return np.float32(S / (float(B) * float(B)))


def kernel(source: np.ndarray, target: np.ndarray) -> np.ndarray:
    nc = build_program()
    in_maps = prepare_inputs(source, target)
    res = run_bass_kernel_spmd(nc, in_maps, list(range(NCORES)))
    outs = [res.results[c]["out"] for c in range(NCORES)]
    return np.asarray(reduce_outputs(outs), dtype=np.float32)


# revision 9
# speedup vs baseline: 1.0872x; 1.0872x over previous
"""MMD loss kernel for Trainium2, SPMD across 8 NeuronCores.

Math: loss = (1/B^2) * sum_{ij} s_i s_j K_ij over the [2B, 2B] Gaussian
kernel-sum matrix, s = [+1]*B ++ [-1]*B.  K_ij = sum_{k=0..4} exp(-l2_ij / (bw*2^k))
with bw = mean off-diagonal l2 / 4 (computed on host via the algebraic identity
sum(l2) = 2N*sum(sq) - 2*||sum x||^2).

Device strategy per core (SPMD, identical program; per-core data sliced on host):
  - 16x16 grid of 512-wide blocks over the symmetric 8192x8192 matrix.
    Core c owns block-rows {2c, 2c+1}. 17 blocks per core: (r0, d=0..7),
    (r1, d=0..7) and ONE wrap-diagonal d=8 block at weight 2 (cores 0-3 take
    the even row's pair, cores 4-7 the odd row's; v slots 9/10 hold its data
    so the program is identical across cores). d=0 weight 1, d=1..7 weight 2.
  - PSUM accumulates P = x_i.x_j - (sq_i+sq_j)/2 = -l2/2 via 4 fp8e4m3
    DoubleRow matmuls (kd-pair contraction, 2x PE rate: ~216ns warm at N=512)
    plus one K=2 fp32r matmul adding the exact -(sq_i+sq_j)/2 (strip-packed:
    4 concurrent row-group MMs via tile_position, ~470ns).
  - Elementwise: with t1 = exp(2c4*P), the five kernel levels are
    t1^(2^k), k=0..4.  ONE ACT exp produces t1 (bf16) with accum -> S1;
    ONE custom DVE op (POWSUM8_ANT: x^2+x^4+x^8+x^16, accum=ADD, runs in
    the fp32 datapath) produces S2+S4+S8+S16 in a single 1x pass (~2.3us).
    Steady state is PE-bound at ~4us/block (ACT ~2.3us, DVE ~2.4us busy).
  - Head: the aug strips' strided partitions all map to DMA engines 0/1,
    serializing those transfers on one engine each (~6us/114KB); the aug
    DMA is split so a 32KB head piece (all of block 0's columns) lands by
    ~11us and the remainder follows v1. Bridge warmup matmuls keep the PE
    clock up across the first-DMA latency window.
  - Tail: the last block keeps full-width matmuls (quartered matmuls are
    LDWEIGHTS-bound) and halves only the ACT/DVE drain.
  - DMA: whole-slot 512KB transfers (sync engine pays ~600ns per
    DMA_DIRECT2D regardless of size).
  - Host reduces the per-core [128, 36] sums with block weights/signs.
"""

import sys

sys.path.insert(0, "/opt/trn_rl_repo")

import numpy as np
import ml_dtypes

import concourse.mybir as mybir
import concourse.tile as tile
from concourse import bacc
from concourse.bass_utils import run_bass_kernel_spmd

B = 4096
D = 1024
N = 2 * B
NB = 16          # block grid (512-wide)
BS = 512
KD = 8           # feature k-tiles of 128
G = 4            # kd pairs (DoubleRow groups)
NCORES = 8
NVSLOT = 11      # v slots: 0..8 consecutive cols, 9/10 the wrap-pair block
NBLK = 17
NSLOT = 2 * 16 + 2 * 2   # [S1, Spow] per full block; last block halved

FP8 = mybir.dt.float8e4
BF16 = mybir.dt.bfloat16
F32 = mybir.dt.float32
F32R = mybir.dt.float32r
NP_FP8 = ml_dtypes.float8_e4m3

# program block list: (lhsT slot, rhs slot)
BLOCKS = [(0, d) for d in range(8)] + [(1, 1 + d) for d in range(8)] + [(9, 10)]
UACOL = {0: 0, 1: 1, 9: 2}   # lhsT slot -> aug column holding -sq/2

_prog_cache = {}


def register_powsum():
    """Register POWSUM8_ANT (x^2+x^4+x^8+x^16, accum=ADD) in dve_ops.OPS.

    The uops sha is computed at registration so the pin check always
    matches this toolchain's lower() output."""
    import concourse.dve_ops as dve_ops
    from concourse.dve_spec import Spec, Src0, sq, lower, AluOp
    from concourse.dve_uop import DveOpSpec

    for o in dve_ops.OPS:
        if o.name == "POWSUM8_ANT":
            return o

    x = Src0
    m1 = sq(x)
    m2 = sq(m1)
    m3 = sq(m2)
    m4 = sq(m3)

    def ref(in0, in1, s0, s1, imm2):
        b = in0.astype(np.float32)
        b2 = b * b
        b4 = b2 * b2
        b8 = b4 * b4
        b16 = b8 * b8
        o = (b2 + b4 + b8 + b16).astype(np.float32)
        return o, o.reshape(o.shape[0], -1).sum(-1, keepdims=True)

    spec = Spec(body=m1 + m2 + m3 + m4, accum=AluOp.ADD, reference=ref)
    name = "POWSUM8_ANT"
    row = dve_ops._CUSTOM_DVE_ROW_BASE + len(dve_ops.OPS)
    assert row < 0x20
    shas = {}
    for ver in ("v3", "v4"):
        try:
            u = lower(spec, ver=ver)
            shas[ver] = DveOpSpec(name=name, opcode=row, uops=u, rd1_en=False).sha(ver)
        except Exception:
            pass
    op = dve_ops.DveOp(name, spec, subdim=False, uops_sha=shas)
    dve_ops.OPS.append(op)
    dve_ops._SUB_OPCODE_FOR_NAME[name] = row
    dve_ops.CUSTOM_DVE_SPECS[name] = spec
    return op


def build_program():
    if "nc" in _prog_cache:
        return _prog_cache["nc"]
    powsum = register_powsum()
    nc = bacc.Bacc("TRN2", target_bir_lowering=False, debug=False, num_devices=NCORES)
    v_d = nc.dram_tensor("v", [NVSLOT, 128, KD, BS], FP8, kind="ExternalInput").ap()
    aug_d = nc.dram_tensor("aug", [8, 14, BS], F32R, kind="ExternalInput").ap()
    sc_d = nc.dram_tensor("sc", [128, 3], F32, kind="ExternalInput").ap()
    out_d = nc.dram_tensor("out", [128, NSLOT], F32, kind="ExternalOutput").ap()

    EXP = mybir.ActivationFunctionType.Exp
    DR = mybir.MatmulPerfMode.DoubleRow

    with tile.TileContext(nc) as tc:
        with (
            tc.tile_pool(name="vstat", bufs=1) as vpool,
            tc.tile_pool(name="augp", bufs=1) as augpool,
            tc.tile_pool(name="cst", bufs=1) as cstpool,
            tc.tile_pool(name="ot", bufs=1) as opool,
            tc.tile_pool(name="texp", bufs=3) as tpool,
            tc.tile_pool(name="wsq", bufs=2) as wpool,
            tc.tile_pool(name="ps", bufs=2, space="PSUM") as pspool,
        ):
            v_sb = vpool.tile([128, NVSLOT, KD, BS], FP8)
            aug_sb = augpool.tile([128, 14, BS], F32R)
            sc_sb = cstpool.tile([128, 3], F32)
            out_sb = opool.tile([128, NSLOT], F32)

            # DMA order == consumption order, with one trap: the aug strips'
            # strided dst partitions (32g+k) all map to DMA engine k (p%16),
            # so each aug transfer serializes onto ONE engine (~6us for
            # 114KB). Split each into a head piece (cols 0:4 = all three ua
            # rows + ones/va slot 0 -> everything block 0 touches, ~32KB,
            # ~2us) right after v0 chunk 0, and the remainder (cols 4:14)
            # after v1 (block b first touches col 3+b at ~14+4b us).
            # aug strips: partitions 32g+0 get (ua=-sq/2, va=+1) rows,
            # partitions 32g+1 get (ua=-1, va=sq/2) rows.
            nc.sync.dma_start(
                out=v_sb[:, 0, 0:2, :], in_=v_d[0, :, 0:2, :],
            )
            nc.sync.dma_start(out=aug_sb[0:128:32, 0:4, :], in_=aug_d[0:4, 0:4, :])
            nc.sync.dma_start(out=aug_sb[1:128:32, 0:4, :], in_=aug_d[4:8, 0:4, :])
            nc.sync.dma_start(out=sc_sb[:], in_=sc_d[:])
            for g in range(1, G):
                nc.sync.dma_start(
                    out=v_sb[:, 0, 2 * g:2 * g + 2, :],
                    in_=v_d[0, :, 2 * g:2 * g + 2, :],
                )
            for h in range(2):
                nc.sync.dma_start(
                    out=v_sb[:, 1, 4 * h:4 * h + 4, :],
                    in_=v_d[1, :, 4 * h:4 * h + 4, :],
                )
            nc.sync.dma_start(out=aug_sb[0:128:32, 4:14, :], in_=aug_d[0:4, 4:14, :])
            nc.sync.dma_start(out=aug_sb[1:128:32, 4:14, :], in_=aug_d[4:8, 4:14, :])
            for s in range(2, NVSLOT):
                nc.sync.dma_start(out=v_sb[:, s], in_=v_d[s])

            # PE warm-up: 56 junk matmuls (as tuned previously) plus a few
            # larger bridge matmuls that keep the HAM activity window busy
            # until the first v0 chunk lands (~10.2us); otherwise the clock
            # re-gates to 1.2GHz and the first two blocks run at half rate.
            # Single upfront memset: a second memset after the small warmups
            # would WAR-depend on all of them and stall the bridge ~1.4us.
            wtile = cstpool.tile([128, 512], BF16)
            nc.gpsimd.memset(wtile[:], 0.25)
            ps_w = pspool.tile([128, 4, BS], F32, name="ps_warm", tag="ps")
            for _ in range(56):
                nc.tensor.matmul(
                    ps_w[0:8, 0, 0:8],
                    lhsT=wtile[:, 0:8],
                    rhs=wtile[:, 0:8],
                    start=True,
                    stop=True,
                )
            # bridge warmups: span the remaining first-DMA latency window
            # (~8.9-11.5us at the mid-clock 427ns each) so the PE clock
            # doesn't re-gate before real data lands.
            for _ in range(6):
                nc.tensor.matmul(
                    ps_w[0:8, 0, :],
                    lhsT=wtile[:, 0:8],
                    rhs=wtile[:, :],
                    start=True,
                    stop=True,
                )

            def emit_mms(ps, lb, rb, cs, cp):
                for g in range(G):
                    for it in range(4):
                        nc.tensor.matmul(
                            ps[:, it, cp],
                            lhsT=v_sb[:, lb, 2 * g:2 * g + 2, it * 128:(it + 1) * 128],
                            rhs=v_sb[:, rb, 2 * g:2 * g + 2, cs],
                            start=(g == 0),
                            stop=False,
                            perf_mode=DR,
                        )
                for it in range(4):
                    nc.tensor.matmul(
                        ps[:, it, cp],
                        lhsT=aug_sb[32 * it:32 * it + 2, UACOL[lb], it * 128:(it + 1) * 128],
                        rhs=aug_sb[32 * it:32 * it + 2, 3 + rb, cs],
                        start=False,
                        stop=True,
                        tile_position=(32 * it, 0),
                    )

            def emit_elem(ps, t1, junk, cs, cp, sbase):
                # t1 = exp(2*c4*P), accum -> S1; POWSUM8 -> S2+S4+S8+S16
                nc.scalar.activation(
                    t1[:, :, cs], ps[:, :, cp], EXP,
                    scale=sc_sb[:, 0:1],
                    accum_out=out_sb[:, sbase:sbase + 1],
                )
                nc.vector._custom_dve(
                    powsum,
                    out=junk[:, :, cs],
                    in0=t1[:, :, cs],
                    accum_out=out_sb[:, sbase + 1:sbase + 2],
                )

            full = slice(0, BS)
            for b, (lb, rb) in enumerate(BLOCKS):
                last = b == NBLK - 1
                t1 = tpool.tile([128, 4, BS], BF16, name=f"t1_{b}", tag="t1")
                junk = wpool.tile([128, 4, BS], BF16, name=f"junk_{b}", tag="junk")
                if not last:
                    ps = pspool.tile([128, 4, BS], F32, name=f"ps_{b}", tag="ps")
                    emit_mms(ps, lb, rb, full, full)
                    emit_elem(ps, t1, junk, full, full, 2 * b)
                else:
                    # full-width matmuls (quartered matmuls are LDWEIGHTS-
                    # bound and slower); only the elementwise drain is halved
                    # so ACT/DVE pipeline the tail.
                    ps = pspool.tile([128, 4, BS], F32, name=f"ps_{b}", tag="ps")
                    emit_mms(ps, lb, rb, full, full)
                    for h in range(2):
                        hs = slice(h * 256, (h + 1) * 256)
                        emit_elem(ps, t1, junk, hs, hs, 32 + 2 * h)
                if b == 15:
                    nc.sync.dma_start(out=out_d[:, 0:32], in_=out_sb[:, 0:32])
            nc.sync.dma_start(out=out_d[:, 32:NSLOT], in_=out_sb[:, 32:NSLOT])
    nc.compile()
    _prog_cache["nc"] = nc
    return nc


def core_slots(c):
    """Global 512-col block indices held by v slots 0..10 on core c."""
    a0 = 2 * c
    slots = [(a0 + s) % NB for s in range(9)]
    if c < 4:
        slots += [a0 % NB, (a0 + 8) % NB]
    else:
        slots += [(a0 + 1) % NB, (a0 + 9) % NB]
    return slots


def prepare_inputs(source: np.ndarray, target: np.ndarray):
    """Host-side shard prep. Returns in_maps for the 8 cores."""
    total = np.concatenate([source, target], axis=0).astype(np.float32)  # [N, D]
    t64 = total.astype(np.float64)
    sq64 = np.einsum("nd,nd->n", t64, t64)
    S1 = sq64.sum()
    vsum = t64.sum(axis=0)
    sum_l2 = 2.0 * N * S1 - 2.0 * (vsum @ vsum)
    bandwidth = sum_l2 / (N * N - N)
    bandwidth = bandwidth / (2.0 ** (5 // 2))  # KERNEL_MUL ** (KERNEL_NUM // 2)
    c4 = np.float64(1.0) / (16.0 * bandwidth)

    sq32 = sq64.astype(np.float32)
    Tt = np.ascontiguousarray(total.T)  # [D, N] f32
    v_all = Tt.astype(NP_FP8).reshape(KD, 128, N)

    # psum holds -l2/2, so the exp scale is doubled vs the -l2 formulation
    sc_np = np.empty((128, 3), dtype=np.float32)
    sc_np[:, 0] = np.float32(2.0 * c4)
    sc_np[:, 1] = np.float32(4.0 * c4)
    sc_np[:, 2] = np.float32(8.0 * c4)

    in_maps = []
    for c in range(NCORES):
        slots = core_slots(c)
        v_np = np.empty((NVSLOT, 128, KD, BS), dtype=NP_FP8)
        aug_np = np.zeros((8, 14, BS), dtype=np.float32)
        for s, gcol in enumerate(slots):
            cols = slice(gcol * BS, (gcol + 1) * BS)
            v_np[s] = v_all[:, :, cols].transpose(1, 0, 2)
        for g in range(4):
            for li, sl in enumerate([slots[0], slots[1], slots[9]]):
                aug_np[g, li] = -0.5 * sq32[sl * BS:(sl + 1) * BS]
            aug_np[g, 3:14] = 1.0
            aug_np[4 + g, 0:3] = -1.0
            for s, gcol in enumerate(slots):
                aug_np[4 + g, 3 + s] = 0.5 * sq32[gcol * BS:(gcol + 1) * BS]
        in_maps.append({"v": v_np, "aug": aug_np, "sc": sc_np})
    return in_maps


def reduce_outputs(outs):
    """outs: list of [128, NSLOT] f32 per core -> loss (np.float32 scalar)."""
    S = 0.0
    for c in range(NCORES):
        o = outs[c].astype(np.float64)  # [128, NSLOT]
        cols = o.sum(axis=0)  # [NSLOT]
        slots = core_slots(c)
        for b, (lb, rb) in enumerate(BLOCKS):
            if b < 16:
                bases = [2 * b]
            else:
                bases = [32 + 2 * h for h in range(2)]
            bsum = sum(cols[base] + cols[base + 1] for base in bases)
            grow = slots[lb]
            gcol = slots[rb]
            w = 1.0 if (b == 0 or b == 8) else 2.0
            sr = 1.0 if grow < NB // 2 else -1.0
            sg = 1.0 if gcol < NB // 2 else -1.0
            S += w * sr * sg * bsum
    return np.float32(S / (float(B) * float(B)))


def kernel(source: np.ndarray, target: np.ndarray) -> np.ndarray:
    nc = build_program()
    in_maps = prepare_inputs(source, target)
    res = run_bass_kernel_spmd(nc, in_maps, list(range(NCORES)))
    outs = [res.results[c]["out"] for c in range(NCORES)]
    return np.asarray(reduce_outputs(outs), dtype=np.float32)
